# revision 1
# baseline (speedup 1.0000x reference)
"""Trainium2 Bass kernel: causal multi-head attention with RoPE (B=1, S=4096,
D=768, H=12) distributed over 8 NeuronCores.

Sharding strategy
-----------------
- Q rows are strided across cores (core c owns rows r = c mod 8).  Causal work
  is then uniform across cores, which is required because the SPMD program is
  identical on every core.
- K/V projections are computed on contiguous 512-row shards per core, RoPE'd
  and transposed locally, then AllGather'd (bf16) so every core holds full K/V.
- Attention runs in "transposed scores" layout: S^T[k, q] = K_rope @ Q_rope^T
  so that the AV matmul consumes exp(S^T) directly (no P transposes) and a
  ones-column appended to V yields the softmax denominators in the same
  accumulation.  Softmax is computed without max-subtraction (scores ~N(0,1)).
- RoPE pairs are de-interleaved by permuting W_q/W_k columns host-side so the
  rotation is a full-width unit-stride vector op.
"""

import math
import os
import sys

import numpy as np

sys.path.insert(0, "/opt/trn_rl_repo")

import ml_dtypes

import concourse.bass as bass
import concourse.mybir as mybir
import concourse.tile as tile
from concourse import bacc
from concourse.bass_utils import run_bass_kernel_spmd
from concourse.masks import make_identity

BF = ml_dtypes.bfloat16
F32 = mybir.dt.float32
BF16 = mybir.dt.bfloat16

S, D, H, DH = 4096, 768, 12, 64
NC = 8
SL = S // NC          # 512 rows per core (both q-strided and kv-contiguous)
NJ = SL // 128        # 4 q-tiles per head per core
NM = S // 128         # 32 k-tiles
NDC = D // 128        # 6 contraction chunks
EXPB = 3              # k-tiles per exp batch
DMAT = True


def build_nc():
    nc = bacc.Bacc(None, target_bir_lowering=False, debug=False)

    xq_t = nc.dram_tensor("xq_t", [D, SL], BF16, kind="ExternalInput")
    xkv_t = nc.dram_tensor("xkv_t", [D, SL], BF16, kind="ExternalInput")
    wq = nc.dram_tensor("wq", [D, D], BF16, kind="ExternalInput")
    wk = nc.dram_tensor("wk", [D, D], BF16, kind="ExternalInput")
    wv = nc.dram_tensor("wv", [D, D], BF16, kind="ExternalInput")
    wo = nc.dram_tensor("wo", [D, D], BF16, kind="ExternalInput")
    cosq = nc.dram_tensor("cosq", [SL, H * 32], BF16, kind="ExternalInput")
    sinq = nc.dram_tensor("sinq", [SL, H * 32], BF16, kind="ExternalInput")
    cosk = nc.dram_tensor("cosk", [SL, H * 32], BF16, kind="ExternalInput")
    sink = nc.dram_tensor("sink", [SL, H * 32], BF16, kind="ExternalInput")
    mask8 = nc.dram_tensor("mask8", [128, 8 * 128], BF16, kind="ExternalInput")
    y_d = nc.dram_tensor("y", [SL, D], F32, kind="ExternalOutput")

    with tile.TileContext(nc) as tc:
        # ---- persistent pool (lives to the end) ----
        P1 = tc.alloc_tile_pool(name="persist", bufs=1)
        wo_sb = P1.tile([128, NDC, D], BF16)
        mk_sb = P1.tile([128, 8, 128], BF16)
        ident = P1.tile([128, 128], BF16)
        make_identity(nc, ident)
        qt_sb = P1.tile([128, NDC, SL], BF16)     # Q_rope^T, local
        att_sb = P1.tile([128, NDC, SL], BF16)    # attention out^T (normalized)
        ktg_sb = P1.tile([128, NDC, NM, 128], BF16)   # gathered K_rope^T
        vog_sb = P1.tile([128, NM, H, DH + 1], BF16)  # gathered V (+ones col)

        KT_N = D * SL
        V_N = SL * H * (DH + 1)
        PD = tc.alloc_tile_pool(name="dram", bufs=1, space="DRAM")
        kt_b = PD.tile([KT_N], BF16)
        v_b = PD.tile([V_N], BF16)
        kt_g = PD.tile([NC * KT_N], BF16, addr_space="Shared")
        v_g = PD.tile([NC * V_N], BF16, addr_space="Shared")

        # ---- projection + rope + transpose for one stream ----
        def proj_stream(x_t_sb, w_sb, cos_sb, sin_sb, dst_sb):
            """dst_sb: [128, NDC, SL] transposed rope'd projection."""
            PP = tc.alloc_tile_pool(name="proj_ps", bufs=2, space="PSUM")
            PT = tc.alloc_tile_pool(name="tr_ps", bufs=4, space="PSUM")
            PW = tc.alloc_tile_pool(name="proj_work", bufs=2)
            for st in range(NJ):
                n_ps = PP.tile([128, D], F32, tag="n_ps")
                for dc in range(NDC):
                    lt = x_t_sb[:, dc, st * 128:(st + 1) * 128]
                    nc.tensor.matmul(n_ps[:, 0:512], lt, w_sb[:, dc, 0:512],
                                     start=(dc == 0), stop=(dc == NDC - 1))
                    nc.tensor.matmul(n_ps[:, 512:768], lt, w_sb[:, dc, 512:768],
                                     start=(dc == 0), stop=(dc == NDC - 1))
                # rope in natural layout: per head [x0(32) | x1(32)]
                x0 = n_ps.rearrange("p (h d) -> p h d", h=H)[:, :, 0:32]
                x1 = n_ps.rearrange("p (h d) -> p h d", h=H)[:, :, 32:64]
                cs = cos_sb[:, st].rearrange("p (h d) -> p h d", h=H)
                sn = sin_sb[:, st].rearrange("p (h d) -> p h d", h=H)
                ta = PW.tile([128, H, 32], F32, tag="ta")
                tb = PW.tile([128, H, 32], F32, tag="tb")
                r_sb = PW.tile([128, H, 64], BF16, tag="r_sb")
                nc.vector.tensor_mul(ta, x0, cs)
                nc.vector.tensor_mul(tb, x1, sn)
                nc.vector.tensor_sub(r_sb[:, :, 0:32], ta, tb)
                nc.vector.tensor_mul(ta, x0, sn)
                nc.vector.tensor_mul(tb, x1, cs)
                nc.vector.tensor_add(r_sb[:, :, 32:64], ta, tb)
                # transpose to [dh, s]
                rf = r_sb.rearrange("p h d -> p (h d)")
                for dc in range(NDC):
                    if DMAT:
                        nc.sync.dma_start(
                            out=dst_sb[:, dc, st * 128:(st + 1) * 128],
                            in_=rf[:, dc * 128:(dc + 1) * 128], transpose=True)
                    else:
                        t_ps = PT.tile([128, 128], BF16, tag="t_ps")
                        nc.tensor.transpose(
                            t_ps, rf[:, dc * 128:(dc + 1) * 128], ident)
                        nc.vector.tensor_copy(
                            dst_sb[:, dc, st * 128:(st + 1) * 128], t_ps)
            PW.release()
            PT.release()
            PP.release()

        def v_proj(x_t_sb, v_w_sb, v_dst):
            PP = tc.alloc_tile_pool(name="vproj_ps", bufs=2, space="PSUM")
            for st in range(NJ):
                v_ps = PP.tile([128, D], F32, tag="v_ps")
                for dc in range(NDC):
                    lt = x_t_sb[:, dc, st * 128:(st + 1) * 128]
                    nc.tensor.matmul(v_ps[:, 0:512], lt, v_w_sb[:, dc, 0:512],
                                     start=(dc == 0), stop=(dc == NDC - 1))
                    nc.tensor.matmul(v_ps[:, 512:768], lt,
                                     v_w_sb[:, dc, 512:768],
                                     start=(dc == 0), stop=(dc == NDC - 1))
                nc.vector.tensor_copy(
                    v_dst[:, st, :, 0:DH],
                    v_ps.rearrange("p (h d) -> p h d", h=H))
            PP.release()

        # ---- K/V shard ----
        P2 = tc.alloc_tile_pool(name="kv_in", bufs=1)
        xkv_sb = P2.tile([128, NDC, SL], BF16)
        nc.sync.dma_start(out=xkv_sb, in_=xkv_t.rearrange("(c p) s -> p c s", p=128))
        wk_sb = P2.tile([128, NDC, D], BF16)
        nc.sync.dma_start(out=wk_sb, in_=wk.rearrange("(c p) d -> p c d", p=128))
        wv_sb = P2.tile([128, NDC, D], BF16)
        nc.sync.dma_start(out=wv_sb, in_=wv.rearrange("(c p) d -> p c d", p=128))
        ck_sb = P2.tile([128, NJ, H * 32], BF16)
        nc.sync.dma_start(out=ck_sb, in_=cosk.rearrange("(t p) d -> p t d", p=128))
        sk_sb = P2.tile([128, NJ, H * 32], BF16)
        nc.sync.dma_start(out=sk_sb, in_=sink.rearrange("(t p) d -> p t d", p=128))
        kts_sb = P2.tile([128, NDC, SL], BF16)
        vs_sb = P2.tile([128, NJ, H, DH + 1], BF16)
        nc.vector.memset(vs_sb[:, :, :, DH:DH + 1], 1.0)

        proj_stream(xkv_sb, wk_sb, ck_sb, sk_sb, kts_sb)
        nc.sync.dma_start(
            out=kt_b[:].rearrange("(c p s) -> p c s", p=128, c=NDC),
            in_=kts_sb)
        nc.gpsimd.collective_compute(
            "AllGather", mybir.AluOpType.bypass,
            replica_groups=[list(range(NC))],
            ins=[kt_b[:]], outs=[kt_g[:]],
        )
        v_proj(xkv_sb, wv_sb, vs_sb)
        nc.sync.dma_start(
            out=v_b[:].rearrange("(t p h e) -> p t h e", p=128, t=NJ, h=H),
            in_=vs_sb)
        nc.gpsimd.collective_compute(
            "AllGather", mybir.AluOpType.bypass,
            replica_groups=[list(range(NC))],
            ins=[v_b[:]], outs=[v_g[:]],
        )

        # ---- Q shard (overlaps the collective) ----
        P3 = tc.alloc_tile_pool(name="q_in", bufs=1)
        xq_sb = P3.tile([128, NDC, SL], BF16)
        nc.sync.dma_start(out=xq_sb, in_=xq_t.rearrange("(c p) s -> p c s", p=128))
        wq_sb = P3.tile([128, NDC, D], BF16)
        nc.sync.dma_start(out=wq_sb, in_=wq.rearrange("(c p) d -> p c d", p=128))
        cq_sb = P3.tile([128, NJ, H * 32], BF16)
        nc.sync.dma_start(out=cq_sb, in_=cosq.rearrange("(t p) d -> p t d", p=128))
        sq_sb = P3.tile([128, NJ, H * 32], BF16)
        nc.sync.dma_start(out=sq_sb, in_=sinq.rearrange("(t p) d -> p t d", p=128))

        proj_stream(xq_sb, wq_sb, cq_sb, sq_sb, qt_sb)
        nc.sync.dma_start(out=wo_sb, in_=wo.rearrange("(c p) d -> p c d", p=128))
        nc.sync.dma_start(out=mk_sb, in_=mask8.rearrange("p (m q) -> p m q", m=8))
        P3.release()
        P2.release()

        # ---- load gathered K/V into SBUF caches ----
        ktg_view = kt_g.rearrange("(r c p s) -> r c p s", r=NC, c=NDC, p=128)
        for r in range(NC):
            for dc in range(NDC):
                eng = nc.sync if dc % 2 == 0 else nc.scalar
                eng.dma_start(
                    out=ktg_sb[:, dc, 4 * r:4 * (r + 1), :]
                        .rearrange("p m s -> p (m s)"),
                    in_=ktg_view[r, dc])
        vg_view = v_g.rearrange(
            "(r t p n) -> r p t n", r=NC, t=NJ, p=128)
        for r in range(NC):
            nc.gpsimd.dma_start(
                out=vog_sb[:, 4 * r:4 * (r + 1), :, :]
                    .rearrange("p m h e -> p m (h e)"),
                in_=vg_view[r])

        # ---- attention ----
        PS = tc.alloc_tile_pool(name="sc_ps", bufs=2, space="PSUM")
        PO = tc.alloc_tile_pool(name="o_ps", bufs=2, space="PSUM")
        PA = tc.alloc_tile_pool(name="att_work", bufs=8)
        PRD = tc.alloc_tile_pool(name="rd_dram", bufs=2, space="DRAM")

        for hp in range(H // 2):
            dc = hp
            o_A = PO.tile([DH + 1, SL], F32, tag="o_ps", name="o_A")
            o_B = PO.tile([DH + 1, SL], F32, tag="o_ps", name="o_B")
            for g in range(NJ):
                qoff = g * 128
                ml0 = 0
                while ml0 < 8:
                    # narrow the window: columns left of 16*ml0 are fully
                    # masked for every k-tile in this batch
                    woff = qoff + 16 * ml0
                    wb = SL - woff
                    # slot stride: each member must stay inside one PSUM bank
                    slot = 512 if wb > 256 else (256 if wb > 128 else 128)
                    nb = min(8 - ml0, 1536 // slot)
                    mw = 128 - 16 * ml0
                    sc_A = PS.tile([128, 1536], F32, tag="sc_ps", name="sc_A")
                    sc_B = PS.tile([128, 1536], F32, tag="sc_ps", name="sc_B")
                    p_A = PA.tile([128, 1536], BF16, tag="p_sb", name="p_A")
                    p_B = PA.tile([128, 1536], BF16, tag="p_sb", name="p_B")
                    svA = sc_A.rearrange("p (m q) -> p m q", q=slot)[:, 0:nb, 0:wb]
                    svB = sc_B.rearrange("p (m q) -> p m q", q=slot)[:, 0:nb, 0:wb]
                    pvA = p_A[:, 0:nb * wb].rearrange("p (m q) -> p m q", m=nb)
                    pvB = p_B[:, 0:nb * wb].rearrange("p (m q) -> p m q", m=nb)
                    for mi in range(nb):
                        m = 8 * g + ml0 + mi
                        # even head on PE row-groups 0-1, odd head on 2-3:
                        # alternating emission lets the array run both
                        # concurrently (tile_position from base partition)
                        nc.tensor.matmul(
                            svA[:, mi, :],
                            ktg_sb[0:DH, dc, m, :],
                            qt_sb[0:DH, dc, woff:SL],
                            start=True, stop=True)
                        nc.tensor.matmul(
                            svB[:, mi, :],
                            ktg_sb[DH:128, dc, m, :],
                            qt_sb[DH:128, dc, woff:SL],
                            start=True, stop=True)
                    nc.scalar.activation(
                        pvA, svA, mybir.ActivationFunctionType.Exp, scale=0.125)
                    nc.scalar.activation(
                        pvB, svB, mybir.ActivationFunctionType.Exp, scale=0.125)
                    mks = mk_sb[:, ml0:ml0 + nb, 16 * ml0:128]
                    nc.vector.tensor_mul(pvA[:, :, 0:mw], pvA[:, :, 0:mw], mks)
                    nc.vector.tensor_mul(pvB[:, :, 0:mw], pvB[:, :, 0:mw], mks)
                    for mi in range(nb):
                        m = 8 * g + ml0 + mi
                        nc.tensor.matmul(
                            o_A[:, woff:SL], vog_sb[:, m, 2 * hp, :],
                            pvA[:, mi, :],
                            start=(m == 0), stop=(m == NM - 1))
                        nc.tensor.matmul(
                            o_B[:, woff:SL], vog_sb[:, m, 2 * hp + 1, :],
                            pvB[:, mi, :],
                            start=(m == 0), stop=(m == NM - 1))
                    ml0 += nb
            # normalize: att = o[0:64] * (1/denom) broadcast over partitions
            for po, o_ps in ((0, o_A), (64, o_B)):
                rd = PA.tile([1, SL], F32, tag="rd")
                nc.vector.reciprocal(rd, o_ps[DH:DH + 1, :])
                rd_d = PRD.tile([SL], F32, tag="rd_d")
                nc.sync.dma_start(out=rd_d[None, :], in_=rd)
                b_sb = PA.tile([DH, SL], F32, tag="b_sb")
                nc.sync.dma_start(
                    out=b_sb,
                    in_=bass.AP(tensor=rd_d.tensor, offset=rd_d.offset,
                                ap=[[0, DH], [1, SL]]))
                nc.vector.tensor_mul(
                    att_sb[po:po + DH, dc, :], o_ps[0:DH, :], b_sb)

        PRD.release()
        PA.release()
        PO.release()
        PS.release()

        # ---- output projection ----
        PY = tc.alloc_tile_pool(name="y_ps", bufs=2, space="PSUM")
        PYW = tc.alloc_tile_pool(name="y_work", bufs=2)
        for j in range(NJ):
            y_ps = PY.tile([128, D], F32, tag="y_ps")
            for dc in range(NDC):
                lt = att_sb[:, dc, j * 128:(j + 1) * 128]
                nc.tensor.matmul(y_ps[:, 0:512], lt, wo_sb[:, dc, 0:512],
                                 start=(dc == 0), stop=(dc == NDC - 1))
                nc.tensor.matmul(y_ps[:, 512:768], lt, wo_sb[:, dc, 512:768],
                                 start=(dc == 0), stop=(dc == NDC - 1))
            y_sb = PYW.tile([128, D], F32, tag="y_sb")
            nc.vector.tensor_copy(y_sb, y_ps)
            nc.sync.dma_start(out=y_d[j * 128:(j + 1) * 128, :], in_=y_sb)
        PYW.release()
        PY.release()
        PD.release()
        P1.release()

    nc.compile()
    return nc


_NC_CACHE = None


def _get_nc():
    global _NC_CACHE
    if _NC_CACHE is None:
        _NC_CACHE = build_nc()
    return _NC_CACHE


def make_in_maps(x, rope_freqs, W_q, W_k, W_v, W_o):
    x2 = np.asarray(x, np.float32).reshape(S, D)
    cos = np.cos(np.asarray(rope_freqs, np.float32))
    sin = np.sin(np.asarray(rope_freqs, np.float32))
    perm = np.concatenate(
        [h * 64 + np.concatenate([np.arange(0, 64, 2), np.arange(1, 64, 2)])
         for h in range(H)])
    wq_p = np.asarray(W_q, np.float32)[:, perm].astype(BF)
    wk_p = np.asarray(W_k, np.float32)[:, perm].astype(BF)
    wv_b = np.asarray(W_v, np.float32).astype(BF)
    wo_b = np.asarray(W_o, np.float32).astype(BF)
    xT = np.ascontiguousarray(x2.T)

    in_maps = []
    for c in range(NC):
        xq_t = np.ascontiguousarray(xT[:, c::NC]).astype(BF)
        xkv_t = np.ascontiguousarray(xT[:, SL * c:SL * (c + 1)]).astype(BF)
        cq = np.ascontiguousarray(
            np.broadcast_to(cos[c::NC][:, None, :], (SL, H, 32))).reshape(SL, H * 32).astype(BF)
        sq = np.ascontiguousarray(
            np.broadcast_to(sin[c::NC][:, None, :], (SL, H, 32))).reshape(SL, H * 32).astype(BF)
        ck = np.ascontiguousarray(
            np.broadcast_to(cos[SL * c:SL * (c + 1)][:, None, :],
                            (SL, H, 32))).reshape(SL, H * 32).astype(BF)
        sk = np.ascontiguousarray(
            np.broadcast_to(sin[SL * c:SL * (c + 1)][:, None, :],
                            (SL, H, 32))).reshape(SL, H * 32).astype(BF)
        kr = np.arange(128)[:, None, None]
        ml = np.arange(8)[None, :, None]
        col = np.arange(128)[None, None, :]
        mk = (128 * ml + kr <= 8 * col + c).astype(BF).reshape(128, 8 * 128)
        in_maps.append({
            "xq_t": xq_t, "xkv_t": xkv_t,
            "wq": wq_p, "wk": wk_p, "wv": wv_b, "wo": wo_b,
            "cosq": cq, "sinq": sq, "cosk": ck, "sink": sk,
            "mask8": mk,
        })
    return in_maps


def kernel(x, rope_freqs, W_q, W_k, W_v, W_o):
    nc = _get_nc()
    in_maps = make_in_maps(x, rope_freqs, W_q, W_k, W_v, W_o)
    res = run_bass_kernel_spmd(nc, in_maps, core_ids=list(range(NC)))
    out = np.empty((S, D), np.float32)
    for c in range(NC):
        out[c::NC, :] = res.results[c]["y"]
    return out.reshape(1, S, D)



# revision 22
# speedup vs baseline: 1.1885x; 1.1885x over previous
"""Trainium2 Bass kernel: causal multi-head attention with RoPE (B=1, S=4096,
D=768, H=12) distributed over 8 NeuronCores.

Sharding strategy
-----------------
- Q rows are strided across cores (core c owns rows r = c mod 8) so causal
  work is uniform across cores (the SPMD program is identical on every core).
- K/V projections are computed on contiguous 512-row shards per core, RoPE'd
  and transposed locally, then AllGather'd so every core holds full K/V.
- Attention runs in "transposed scores" layout: S^T[k, q] = K_rope @ Q_rope^T
  so the AV matmul consumes exp(S^T) directly, and a ones-column appended to V
  yields the softmax denominators in the same accumulation.  Softmax is
  computed without max-subtraction (scores ~N(0,1)).
- All math is bf16 (fp8 q/k quantization alone costs 2.7e-2 relative error —
  over the accuracy gate — so the tensor engine runs bf16 throughout).
- AV matmuls are software-pipelined several batches behind their exp so the
  in-order PE stream never stalls on the later-arriving gathered V.
- RoPE pairs are de-interleaved by permuting W_q/W_k columns host-side so the
  rotation is a full-width unit-stride vector op.
- All DRAM inputs are partition-major contiguous so every load is one
  descriptor per partition.
"""

import os as _os
import sys

import numpy as np

sys.path.insert(0, "/opt/trn_rl_repo")

import ml_dtypes

import concourse.bass as bass
import concourse.mybir as mybir
import concourse.tile as tile
from concourse import bacc

BF = ml_dtypes.bfloat16
F32 = mybir.dt.float32
BF16 = mybir.dt.bfloat16

S, D, H, DH = 4096, 768, 12, 64
NC = 8
SL = S // NC          # 512 rows per core (both q-strided and kv-contiguous)
NJ = SL // 128        # 4 row-tiles per core
NM = S // 128         # 32 k-tiles
NDC = D // 128        # 6 contraction chunks == head pairs
H32 = H * 32          # 384

# Concurrent xbar transposes on two HWDGE queues race on real hardware
# (verified: nondeterministic corruption) — keep them on one queue.
F_T2Q = _os.environ.get("K_T2Q", "0") == "1"
F_LAG = int(_os.environ.get("K_LAG", "9"))       # AV software-pipeline depth
F_WARM = _os.environ.get("K_WARM", "1") == "1"   # PE p-state warmup
F_BC0 = _os.environ.get("K_BC0", "1") == "1"     # stride-0 cos/sin broadcast


def build_nc():
    nc = bacc.Bacc(None, target_bir_lowering=False, debug=False)

    xq_t = nc.dram_tensor("xq_t", [128, NDC * SL], BF16, kind="ExternalInput")
    xkv_t = nc.dram_tensor("xkv_t", [128, NDC * SL], BF16, kind="ExternalInput")
    wq = nc.dram_tensor("wq", [128, NDC * D], BF16, kind="ExternalInput")
    wk = nc.dram_tensor("wk", [128, NDC * D], BF16, kind="ExternalInput")
    wv = nc.dram_tensor("wv", [128, NDC * D], BF16, kind="ExternalInput")
    wo = nc.dram_tensor("wo", [128, NDC * D], BF16, kind="ExternalInput")
    cosq = nc.dram_tensor("cosq", [128, NJ * 32], BF16, kind="ExternalInput")
    sinq = nc.dram_tensor("sinq", [128, NJ * 32], BF16, kind="ExternalInput")
    cosk = nc.dram_tensor("cosk", [128, NJ * 32], BF16, kind="ExternalInput")
    sink = nc.dram_tensor("sink", [128, NJ * 32], BF16, kind="ExternalInput")
    mask8 = nc.dram_tensor("mask8", [128, 8 * 128], BF16, kind="ExternalInput")
    y_d = nc.dram_tensor("y", [SL, D], F32, kind="ExternalOutput")

    KT_N = 128 * NDC * SL             # elements of one core's k^T shard
    V_N = 128 * NJ * H * (DH + 1)

    with tile.TileContext(nc) as tc:
        # ---- persistent pool (lives to the end) ----
        P1 = tc.alloc_tile_pool(name="persist", bufs=1)
        wo_sb = P1.tile([128, NDC, D], BF16)
        mk_sb = P1.tile([128, 8, 128], BF16)
        qt_sb = P1.tile([128, NDC, SL], BF16)         # q^T (rope'd)
        att_sb = P1.tile([128, NDC, SL], BF16)        # attention out^T (normed)
        ktg = P1.tile([128, NC, NDC, SL], BF16)       # gathered k^T, r-outer
        vog = P1.tile([128, NC, NJ, H, DH + 1], BF16)  # gathered V (+ones col)

        PD = tc.alloc_tile_pool(name="dram", bufs=1, space="DRAM")
        kt_b = PD.tile([KT_N], BF16)
        v_b = PD.tile([V_N], BF16)
        kt_g = PD.tile([NC * KT_N], BF16, addr_space="Shared")
        v_g = PD.tile([NC * V_N], BF16, addr_space="Shared")

        # ---- projection + rope + transpose for one stream ----
        # r_sb column order per head: [y0(32) | y1(32)], heads in order, so
        # the per-(st, dc) [128,128] transpose lands chunk dc's two heads on
        # partitions [0:64) / [64:128) — the K=64 score-matmul layout.
        def proj_rope_t(x_sb, w_sb, cos_sb, sin_sb, dst_bf, ps_bufs=2,
                        warm=None):
            PP = tc.alloc_tile_pool(name="proj_ps", bufs=ps_bufs, space="PSUM")
            PW = tc.alloc_tile_pool(name="proj_work", bufs=2)
            if warm is not None and F_WARM:
                w_ps = PP.tile([128, 512], F32, tag="warm")
                for _ in range(9):
                    nc.tensor.matmul(w_ps, warm[:, 0:128], warm,
                                     start=True, stop=True)
            for st in range(NJ):
                n_ps = PP.tile([128, D], F32, tag="n_ps")
                for dc in range(NDC):
                    lt = x_sb[:, dc, st * 128:(st + 1) * 128]
                    nc.tensor.matmul(n_ps[:, 0:512], lt, w_sb[:, dc, 0:512],
                                     start=(dc == 0), stop=(dc == NDC - 1))
                    nc.tensor.matmul(n_ps[:, 512:768], lt, w_sb[:, dc, 512:768],
                                     start=(dc == 0), stop=(dc == NDC - 1))
                nb = PW.tile([128, H, 2, 32], BF16, tag="nb")
                nc.vector.tensor_copy(
                    nb.rearrange("p h x i -> p (h x i)"), n_ps)
                x0 = nb[:, :, 0]
                x1 = nb[:, :, 1]
                c0 = cos_sb[:, st]
                s0 = sin_sb[:, st]
                if F_BC0:
                    cs = bass.AP(tensor=c0.tensor, offset=c0.offset,
                                 ap=[list(c0.ap[0]), [0, H], [1, 32]])
                    sn = bass.AP(tensor=s0.tensor, offset=s0.offset,
                                 ap=[list(s0.ap[0]), [0, H], [1, 32]])
                else:
                    csf = PW.tile([128, H, 32], BF16, tag="csf")
                    snf = PW.tile([128, H, 32], BF16, tag="snf")
                    for h in range(H):
                        nc.vector.tensor_copy(csf[:, h], c0)
                        nc.vector.tensor_copy(snf[:, h], s0)
                    cs, sn = csf, snf
                ta = PW.tile([128, H, 32], BF16, tag="ta")
                tb = PW.tile([128, H, 32], BF16, tag="tb")
                tc2 = PW.tile([128, H, 32], BF16, tag="tc")
                td = PW.tile([128, H, 32], BF16, tag="td")
                r_sb = PW.tile([128, H, 2, 32], BF16, tag="r_sb")
                nc.vector.tensor_mul(ta, x0, cs)
                nc.vector.tensor_mul(tb, x1, sn)
                nc.vector.tensor_sub(r_sb[:, :, 0], ta, tb)
                nc.vector.tensor_mul(tc2, x0, sn)
                nc.vector.tensor_mul(td, x1, cs)
                nc.vector.tensor_add(r_sb[:, :, 1], tc2, td)
                rf = r_sb.rearrange("p h x i -> p (h x i)")
                for dc in range(NDC):
                    eng = nc.sync if dc % 2 == 0 or not F_T2Q else nc.scalar
                    eng.dma_start(
                        out=dst_bf[:, dc, st * 128:(st + 1) * 128],
                        in_=rf[:, dc * 128:(dc + 1) * 128],
                        transpose=True)
            PW.release()
            PP.release()

        def v_proj(x_sb, v_w_sb, v_dst):
            PP = tc.alloc_tile_pool(name="vproj_ps", bufs=2, space="PSUM")
            for st in range(NJ):
                v_ps = PP.tile([128, D], F32, tag="v_ps")
                for dc in range(NDC):
                    lt = x_sb[:, dc, st * 128:(st + 1) * 128]
                    nc.tensor.matmul(v_ps[:, 0:512], lt, v_w_sb[:, dc, 0:512],
                                     start=(dc == 0), stop=(dc == NDC - 1))
                    nc.tensor.matmul(v_ps[:, 512:768], lt,
                                     v_w_sb[:, dc, 512:768],
                                     start=(dc == 0), stop=(dc == NDC - 1))
                nc.vector.tensor_copy(
                    v_dst[:, st, :, 0:DH],
                    v_ps.rearrange("p (h d) -> p h d", h=H))
            PP.release()

        # ---- input loads (K-path inputs first; Q/O loads deferred) ----
        P2 = tc.alloc_tile_pool(name="kv_in", bufs=1)
        wk_sb = P2.tile([128, NDC, D], BF16)
        xkv_sb = P2.tile([128, NDC, SL], BF16)
        HC, HD, HS = NDC // 2, NDC // 2 * D, NDC // 2 * SL
        nc.sync.dma_start(out=wk_sb[:, 0:HC].rearrange("p c d -> p (c d)"),
                          in_=wk[:, 0:HD])
        nc.sync.dma_start(out=xkv_sb[:, 0:HC].rearrange("p c s -> p (c s)"),
                          in_=xkv_t[:, 0:HS])
        nc.sync.dma_start(out=wk_sb[:, HC:].rearrange("p c d -> p (c d)"),
                          in_=wk[:, HD:])
        nc.sync.dma_start(out=xkv_sb[:, HC:].rearrange("p c s -> p (c s)"),
                          in_=xkv_t[:, HS:])
        ck_sb = P2.tile([128, NJ, 32], BF16)
        nc.scalar.dma_start(out=ck_sb.rearrange("p t d -> p (t d)"), in_=cosk[:, :])
        sk_sb = P2.tile([128, NJ, 32], BF16)
        nc.scalar.dma_start(out=sk_sb.rearrange("p t d -> p (t d)"), in_=sink[:, :])
        P3 = tc.alloc_tile_pool(name="q_in", bufs=1)
        cq_sb = P3.tile([128, NJ, 32], BF16)
        nc.scalar.dma_start(out=cq_sb.rearrange("p t d -> p (t d)"), in_=cosq[:, :])
        sq_sb = P3.tile([128, NJ, 32], BF16)
        nc.scalar.dma_start(out=sq_sb.rearrange("p t d -> p (t d)"), in_=sinq[:, :])
        wv_sb = P2.tile([128, NDC, D], BF16)
        nc.sync.dma_start(out=wv_sb.rearrange("p c d -> p (c d)"), in_=wv[:, :])
        wq_sb = P3.tile([128, NDC, D], BF16)
        xq_sb = P3.tile([128, NDC, SL], BF16)
        kts_bf = P2.tile([128, NDC, SL], BF16)
        vs_sb = P2.tile([128, NJ, H, DH + 1], BF16)
        nc.vector.memset(vs_sb[:, :, :, DH:DH + 1], 1.0)
        warm_sb = P2.tile([128, 512], BF16)
        nc.vector.memset(warm_sb, 0.0)

        # ---- K shard (critical path to the AllGather) ----
        proj_rope_t(xkv_sb, wk_sb, ck_sb, sk_sb, kts_bf, warm=warm_sb)
        kb_view = kt_b[:].rearrange("(p c s) -> p c s", p=128, c=NDC)
        for dc in range(NDC):
            nc.sync.dma_start(out=kb_view[:, dc], in_=kts_bf[:, dc])
        nc.gpsimd.collective_compute(
            "AllGather", mybir.AluOpType.bypass,
            replica_groups=[list(range(NC))],
            ins=[kt_b[:]], outs=[kt_g[:]],
        )
        # deferred loads: issued only after the K-path DMAs so they don't
        # crowd the descriptor channel ahead of the first collective
        nc.scalar.dma_start(out=wq_sb.rearrange("p c d -> p (c d)"), in_=wq[:, :])
        nc.scalar.dma_start(out=xq_sb.rearrange("p c s -> p (c s)"), in_=xq_t[:, :])
        nc.scalar.dma_start(out=wo_sb.rearrange("p c d -> p (c d)"), in_=wo[:, :])
        nc.scalar.dma_start(
            out=mk_sb.rearrange("p m q -> p (m q)"), in_=mask8[:, :])

        # ---- V shard (store must land before the V AllGather slot) ----
        v_proj(xkv_sb, wv_sb, vs_sb)
        nc.sync.dma_start(
            out=v_b[:].rearrange("(p n) -> p n", p=128),
            in_=vs_sb.rearrange("p t h e -> p (t h e)"))
        nc.gpsimd.collective_compute(
            "AllGather", mybir.AluOpType.bypass,
            replica_groups=[list(range(NC))],
            ins=[v_b[:]], outs=[v_g[:]],
        )

        # ---- Q shard (overlaps the collectives) ----
        # The score-psum pool is allocated BEFORE the Q projection (which
        # runs with a single psum buffer) so the first QK matmuls only wait
        # on the gathered K, not on any projection's PSUM release.
        PS = tc.alloc_tile_pool(name="sc_ps", bufs=2, space="PSUM")
        proj_rope_t(xq_sb, wq_sb, cq_sb, sq_sb, qt_sb, ps_bufs=1)
        P3.release()
        P2.release()

        # ---- load gathered K/V into SBUF caches ----
        ktg_view = kt_g.rearrange("(r p n) -> r p n", r=NC, p=128)
        for r in range(NC):
            nc.sync.dma_start(
                out=ktg[:, r].rearrange("p c s -> p (c s)"),
                in_=ktg_view[r])
        vg_view = v_g.rearrange("(r p n) -> r p n", r=NC, p=128)
        for r in range(NC):
            nc.gpsimd.dma_start(
                out=vog[:, r].rearrange("p t h e -> p (t h e)"),
                in_=vg_view[r])

        # ---- attention ----
        PO = tc.alloc_tile_pool(name="o_ps", bufs=2, space="PSUM")
        PA = tc.alloc_tile_pool(name="att_work", bufs=22)
        PB = tc.alloc_tile_pool(name="bc_work", bufs=2)
        PRD = tc.alloc_tile_pool(name="rd_dram", bufs=2, space="DRAM")

        # AV matmuls are emitted LAG batches behind their exp so the in-order
        # PE stream never stalls on the (later-arriving) gathered V.
        LAG = F_LAG
        pend = []          # (emit_av_closure, normalize_closure_or_None)

        def flush(n):
            while len(pend) > n:
                av, fin = pend.pop(0)
                av()
                if fin is not None:
                    fin()

        for hp in range(H // 2):
            o_A = PO.tile([DH + 1, SL], F32, tag="o_ps", name="o_A")
            o_B = PO.tile([DH + 1, SL], F32, tag="o_ps", name="o_B")
            for g in range(NJ):
                qoff = g * 128
                ml0 = 0
                while ml0 < 8:
                    woff = qoff + 16 * ml0
                    wb = SL - woff
                    # slot stride: each member must stay inside one PSUM bank
                    slot = 512 if wb > 256 else (256 if wb > 128 else 128)
                    nb = min(8 - ml0, 1536 // slot)
                    mw = 128 - 16 * ml0
                    sc_A = PS.tile([128, 1536], F32, tag="sc_ps", name="sc_A")
                    sc_B = PS.tile([128, 1536], F32, tag="sc_ps", name="sc_B")
                    p_A = PA.tile([128, 1536], BF16, tag="p_sb", name="p_A")
                    p_B = PA.tile([128, 1536], BF16, tag="p_sb", name="p_B")
                    svA = sc_A.rearrange("p (m q) -> p m q", q=slot)[:, 0:nb, 0:wb]
                    svB = sc_B.rearrange("p (m q) -> p m q", q=slot)[:, 0:nb, 0:wb]
                    pvA = p_A[:, 0:nb * wb].rearrange("p (m q) -> p m q", m=nb)
                    pvB = p_B[:, 0:nb * wb].rearrange("p (m q) -> p m q", m=nb)
                    for mi in range(nb):
                        m = 8 * g + ml0 + mi
                        r, j = m // 4, m % 4
                        nc.tensor.matmul(
                            svA[:, mi, :],
                            ktg[0:DH, r, hp, j * 128:(j + 1) * 128],
                            qt_sb[0:DH, hp, woff:SL],
                            start=True, stop=True)
                        nc.tensor.matmul(
                            svB[:, mi, :],
                            ktg[DH:128, r, hp, j * 128:(j + 1) * 128],
                            qt_sb[DH:128, hp, woff:SL],
                            start=True, stop=True)
                    nc.scalar.activation(
                        pvA, svA, mybir.ActivationFunctionType.Exp, scale=0.125)
                    nc.scalar.activation(
                        pvB, svB, mybir.ActivationFunctionType.Exp, scale=0.125)
                    mks = mk_sb[:, ml0:ml0 + nb, 16 * ml0:128]
                    nc.vector.tensor_mul(pvA[:, :, 0:mw], pvA[:, :, 0:mw], mks)
                    nc.vector.tensor_mul(pvB[:, :, 0:mw], pvB[:, :, 0:mw], mks)

                    def av(hp=hp, g=g, ml0=ml0, nb=nb, woff=woff,
                           pvA=pvA, pvB=pvB, o_A=o_A, o_B=o_B):
                        for mi in range(nb):
                            m = 8 * g + ml0 + mi
                            r, j = m // 4, m % 4
                            nc.tensor.matmul(
                                o_A[:, woff:SL], vog[:, r, j, 2 * hp, :],
                                pvA[:, mi, :],
                                start=(m == 0), stop=(m == NM - 1))
                            nc.tensor.matmul(
                                o_B[:, woff:SL], vog[:, r, j, 2 * hp + 1, :],
                                pvB[:, mi, :],
                                start=(m == 0), stop=(m == NM - 1))

                    pend.append((av, None))
                    flush(LAG)
                    ml0 += nb

            def norm(hp=hp, o_A=o_A, o_B=o_B):
                # att = o[0:64] * (1/denom) broadcast over partitions
                for po, o_ps in ((0, o_A), (64, o_B)):
                    rd = PB.tile([1, SL], F32, tag="rd")
                    nc.vector.reciprocal(rd, o_ps[DH:DH + 1, :])
                    rd_d = PRD.tile([SL], F32, tag="rd_d")
                    nc.sync.dma_start(out=rd_d[None, :], in_=rd)
                    b_sb = PB.tile([DH, SL], F32, tag="b_sb")
                    nc.sync.dma_start(
                        out=b_sb,
                        in_=bass.AP(tensor=rd_d.tensor, offset=rd_d.offset,
                                    ap=[[0, DH], [1, SL]]))
                    nc.vector.tensor_mul(
                        att_sb[po:po + DH, hp, :], o_ps[0:DH, :], b_sb)

            # attach the normalize to the last AV batch of this head pair
            if pend:
                av_last, _ = pend[-1]
                pend[-1] = (av_last, norm)
            else:
                norm()
        flush(0)

        PRD.release()
        PB.release()
        PA.release()
        PO.release()

        # ---- output projection ----
        PS.release()
        PY = tc.alloc_tile_pool(name="y_ps", bufs=2, space="PSUM")
        PYW = tc.alloc_tile_pool(name="y_work", bufs=2)
        for j in range(NJ):
            y_ps = PY.tile([128, D], F32, tag="y_ps")
            for dc in range(NDC):
                lt = att_sb[:, dc, j * 128:(j + 1) * 128]
                nc.tensor.matmul(y_ps[:, 0:512], lt, wo_sb[:, dc, 0:512],
                                 start=(dc == 0), stop=(dc == NDC - 1))
                nc.tensor.matmul(y_ps[:, 512:768], lt, wo_sb[:, dc, 512:768],
                                 start=(dc == 0), stop=(dc == NDC - 1))
            y_sb = PYW.tile([128, D], F32, tag="y_sb")
            nc.vector.tensor_copy(y_sb, y_ps)
            nc.sync.dma_start(out=y_d[j * 128:(j + 1) * 128, :], in_=y_sb)
        PYW.release()
        PY.release()
        PD.release()
        P1.release()

    nc.compile()
    return nc


_NC_CACHE = None


def _get_nc():
    global _NC_CACHE
    if _NC_CACHE is None:
        _NC_CACHE = build_nc()
    return _NC_CACHE


def _col_perm():
    """W_q/W_k column permutation: per head, de-interleave rope pairs into
    [x0(32) | x1(32)] blocks so the rotation is a unit-stride vector op."""
    return np.concatenate(
        [h * 64 + np.concatenate([np.arange(0, 64, 2), np.arange(1, 64, 2)])
         for h in range(H)])


def _pmajor(w):
    """[D, D] -> [128, NDC*D] partition-major contiguous."""
    return np.ascontiguousarray(
        w.reshape(NDC, 128, D).transpose(1, 0, 2).reshape(128, NDC * D))


def make_in_maps(x, rope_freqs, W_q, W_k, W_v, W_o):
    x2 = np.asarray(x, np.float32).reshape(S, D)
    cos = np.cos(np.asarray(rope_freqs, np.float32)).astype(BF)
    sin = np.sin(np.asarray(rope_freqs, np.float32)).astype(BF)
    perm = _col_perm()
    wq_p = _pmajor(np.asarray(W_q, np.float32)[:, perm].astype(BF))
    wk_p = _pmajor(np.asarray(W_k, np.float32)[:, perm].astype(BF))
    wv_b = _pmajor(np.asarray(W_v, np.float32).astype(BF))
    wo_b = _pmajor(np.asarray(W_o, np.float32).astype(BF))
    xT = x2.T.astype(BF)                       # [D, S]
    xq_all = xT.reshape(NDC, 128, SL, NC)      # [:, :, s, c] = strided q rows
    xkv_all = xT.reshape(NDC, 128, NC, SL)

    # cos/sin: [S, 32] -> [128, NJ*32] (heads broadcast on device, stride-0)
    def rope_tab(tab, rows):
        tt = tab[rows].reshape(NJ, 128, 32)
        return np.ascontiguousarray(tt.transpose(1, 0, 2)).reshape(128, NJ * 32)

    kr = np.arange(128)[:, None, None]
    ml = np.arange(8)[None, :, None]
    col = np.arange(128)[None, None, :]
    in_maps = []
    for c in range(NC):
        xq_t = np.ascontiguousarray(
            xq_all[:, :, :, c].transpose(1, 0, 2)).reshape(128, NDC * SL)
        xkv_t = np.ascontiguousarray(
            xkv_all[:, :, c, :].transpose(1, 0, 2)).reshape(128, NDC * SL)
        qrows = np.arange(SL) * NC + c
        krows = np.arange(SL * c, SL * (c + 1))
        mk = (128 * ml + kr <= 8 * col + c).astype(BF).reshape(128, 8 * 128)
        in_maps.append({
            "xq_t": xq_t, "xkv_t": xkv_t,
            "wq": wq_p, "wk": wk_p, "wv": wv_b, "wo": wo_b,
            "cosq": rope_tab(cos, qrows), "sinq": rope_tab(sin, qrows),
            "cosk": rope_tab(cos, krows), "sink": rope_tab(sin, krows),
            "mask8": mk,
        })
    return in_maps


_EXEC_CACHE = None


def _get_exec():
    """Cached jitted PJRT executable for the compiled Bass module (the stock
    run path re-traces and re-compiles the XLA wrapper on every call)."""
    global _EXEC_CACHE
    if _EXEC_CACHE is not None:
        return _EXEC_CACHE
    import jax
    from jax.sharding import Mesh, PartitionSpec
    from jax.experimental.shard_map import shard_map
    from concourse import bass2jax

    nc = _get_nc()
    bass2jax.install_neuronx_cc_hook()
    pname = nc.partition_id_tensor.name if nc.partition_id_tensor else None
    in_names, out_names, out_avals, zero_outs = [], [], [], []
    for alloc in nc.m.functions[0].allocations:
        if not isinstance(alloc, bass2jax.mybir.MemoryLocationSet):
            continue
        name = alloc.memorylocations[0].name
        if alloc.kind == "ExternalInput":
            if name != pname:
                in_names.append(name)
        elif alloc.kind == "ExternalOutput":
            shape = tuple(alloc.tensor_shape)
            dtype = bass2jax.mybir.dt.np(alloc.dtype)
            out_avals.append(jax.core.ShapedArray(shape, dtype))
            out_names.append(name)
            zero_outs.append(
                np.zeros((NC * shape[0], *shape[1:]), dtype))
    n_params = len(in_names)
    all_names = in_names + out_names
    if pname is not None:
        all_names = all_names + [pname]

    def _body(*args):
        operands = list(args)
        if pname is not None:
            operands.append(bass2jax.partition_id_tensor())
        outs = bass2jax._bass_exec_p.bind(
            *operands, out_avals=tuple(out_avals), in_names=tuple(all_names),
            out_names=tuple(out_names), lowering_input_output_aliases=(),
            sim_require_finite=True, sim_require_nnan=True, nc=nc)
        return tuple(outs)

    devices = jax.devices()[:NC]
    mesh = Mesh(np.asarray(devices), ("core",))
    specs = (PartitionSpec("core"),) * (n_params + len(out_names))
    fn = jax.jit(shard_map(_body, mesh=mesh, in_specs=specs,
                           out_specs=(PartitionSpec("core"),) * len(out_names),
                           check_rep=False))
    zeros_dev = [jax.device_put(z) for z in zero_outs]
    _EXEC_CACHE = (fn, in_names, n_params, zeros_dev)
    return _EXEC_CACHE


def kernel(x, rope_freqs, W_q, W_k, W_v, W_o):
    fn, in_names, n_params, zeros_dev = _get_exec()
    in_maps = make_in_maps(x, rope_freqs, W_q, W_k, W_v, W_o)
    concat_in = [
        np.concatenate([np.asarray(in_maps[c][nm]) for c in range(NC)], 0)
        for nm in in_names
    ]
    out_arrs = fn(*concat_in, *zeros_dev)
    y = np.asarray(out_arrs[0]).reshape(NC, SL, D)
    out = np.empty((S, D), np.float32)
    for c in range(NC):
        out[c::NC, :] = y[c]
    return out.reshape(1, S, D)
